# revision 2
# baseline (speedup 1.0000x reference)
"""Multi-head attention forward on 8 Trainium2 NeuronCores (Bass/Tile), v2.

Problem: B=4, L=2048, D=1024, H=16 heads, DV=64.
  out = softmax((x_q Wq^T + bq)(x_k Wk^T)^T / sqrt(DV)) (x_v Wv^T) Wc^T + const
(bk dropped exactly via softmax column-invariance; bv/bc folded into a
host-side constant row added after the kernel.)

Sharding (8 cores): core c handles batch b = c//2 and head-group g = c%2
(8 heads). Host sums the two per-batch partials and adds the constant row.

All matmuls bf16 (full-rate on the PE regardless of K=64 / M=65 / N=65,
unlike fp32r). Per core:
  A. K/Q projections for pair 0 upfront; V projection and the remaining
     Q/K pairs are interleaved into stage B to keep the PE dense while
     the ACT engine (the near-bottleneck, ~340us of exp) drains scores.
  B. Per unit (pair m, head h2, q-half qc): 16 k-tiles: scores^T
     [k=128, q=1024] in PSUM -> ACT exp (scale=1/8) -> ex bf16; AV in
     [q, d] orientation: lhsT=ex slice [128,128], rhs=vext [128,65]
     (65th col = ones -> softmax denominator in PSUM col 64), N=65
     matmuls accumulate over k into packed PSUM banks (4 q-tiles x 65
     cols per bank). Normalize: per-partition reciprocal + scale on DVE.
     At pair completion: PE-transpose attn [q,d] -> attnT [d,q] bf16.
  C. Output projection from attnT (N=512 matmuls), ACT copies, DMA out.
"""

from collections import deque
from contextlib import ExitStack

import numpy as np

import concourse.bacc as bacc
import concourse.mybir as mybir
from concourse.tile import TileContext
from concourse.bass_utils import run_bass_kernel_spmd

B, L, D, H = 4, 2048, 1024, 16
DV = 64
HPC = 8           # heads per core
OC = HPC * DV     # 512 projection cols per core
NCORES = 8

F32 = mybir.dt.float32
BF16 = mybir.dt.bfloat16
EXP = mybir.ActivationFunctionType.Exp

NI = D // 128     # 8 contraction tiles for projections
NM = HPC // 2     # 4 head pairs per core
NLT = L // 128    # 16 l/k tiles
NQT = 8           # q-tiles per q-half unit
QW = 1024         # q-half width

_CACHE = {}


def _build():
    nc = bacc.Bacc("TRN2", target_bir_lowering=False, debug=False,
                   num_devices=NCORES)

    xtq = nc.dram_tensor("XTQ", [D, L], BF16, kind="ExternalInput")
    xtk = nc.dram_tensor("XTK", [D, L], BF16, kind="ExternalInput")
    xtv = nc.dram_tensor("XTV", [D, L], BF16, kind="ExternalInput")
    wqt = nc.dram_tensor("WQT", [D, OC], BF16, kind="ExternalInput")
    wkt = nc.dram_tensor("WKT", [D, OC], BF16, kind="ExternalInput")
    wvt = nc.dram_tensor("WVT", [D, OC], BF16, kind="ExternalInput")
    wct = nc.dram_tensor("WCT", [OC, D], BF16, kind="ExternalInput")
    bqd = nc.dram_tensor("BQ", [OC], F32, kind="ExternalInput")
    idt = nc.dram_tensor("IDT", [128, 128], BF16, kind="ExternalInput")
    out = nc.dram_tensor("OUT", [L, D], F32, kind="ExternalOutput")

    with TileContext(nc) as tc:
        stack = ExitStack()
        w_pool = stack.enter_context(tc.tile_pool(name="w", bufs=1))
        x_pool = stack.enter_context(tc.tile_pool(name="x", bufs=1))
        qk_pool = stack.enter_context(tc.tile_pool(name="qk", bufs=1))
        ex_pool = stack.enter_context(tc.tile_pool(name="ex", bufs=1))
        at_pool = stack.enter_context(tc.tile_pool(name="at", bufs=1))
        ob_pool = stack.enter_context(tc.tile_pool(name="ob", bufs=2))
        st_pool = stack.enter_context(tc.tile_pool(name="st", bufs=2,
                                                   space="PSUM"))
        av_pool = stack.enter_context(tc.tile_pool(name="av", bufs=2,
                                                   space="PSUM"))
        pc_pool = stack.enter_context(tc.tile_pool(name="pc", bufs=2,
                                                   space="PSUM"))

        # --- static SBUF tensors ---
        ident = w_pool.tile([128, 128], BF16, tag="idt")
        nc.sync.dma_start(out=ident, in_=idt[:, :])
        bq_t = w_pool.tile([128, NM], F32, tag="bq")
        nc.sync.dma_start(out=bq_t,
                          in_=bqd[:].rearrange("(m p) -> p m", p=128))
        wv_t, wq_t, wk_t = [], [], []
        for i in range(NI):
            t = w_pool.tile([128, OC], BF16, tag="wv", bufs=NI, name="wv_t")
            nc.sync.dma_start(out=t, in_=wvt[i * 128:(i + 1) * 128, :])
            wv_t.append(t)
        for i in range(NI):
            t = w_pool.tile([128, OC], BF16, tag="wk", bufs=NI, name="wk_t")
            nc.sync.dma_start(out=t, in_=wkt[i * 128:(i + 1) * 128, :])
            wk_t.append(t)
        for i in range(NI):
            t = w_pool.tile([128, OC], BF16, tag="wq", bufs=NI, name="wq_t")
            nc.sync.dma_start(out=t, in_=wqt[i * 128:(i + 1) * 128, :])
            wq_t.append(t)
        wc_t = []
        for dt in range(NM):
            t = w_pool.tile([128, D], BF16, tag="wc", bufs=NM, name="wc_t")
            nc.sync.dma_start(out=t, in_=wct[dt * 128:(dt + 1) * 128, :])
            wc_t.append(t)

        # vext: [128(k), 8 heads, 65] bf16, ones in col 64 of each head
        vext = [x_pool.tile([128, HPC, DV + 1], BF16, tag="vext", bufs=NLT,
                            name=f"vext{k}") for k in range(NLT)]
        for k in range(NLT):
            nc.vector.memset(vext[k][:, :, DV], 1.0)

        # kt per pair: [128 (2 heads x 64 dv), L] bf16.
        # Q lives in a 4-slot ring of ZERO-PADDED per-head tiles: head h
        # occupies its own 64 rows (h%2 parity decides which half), the
        # other 64 rows stay zero. Scores then contract over K=128 with
        # the full kt tile as lhsT -- the zero rows contribute nothing,
        # and the full-K shape keeps the PE clock at 2.4 GHz (half-K
        # matmul streams throttle the PE to 1.2 GHz).
        kt = [qk_pool.tile([128, L], BF16, tag="qkt", bufs=NM,
                           name=f"kt{m}") for m in range(NM)]
        qzt = [qk_pool.tile([128, L], BF16, tag="qzt", bufs=4,
                            name=f"qzt{j}") for j in range(4)]
        for j in range(4):
            if j % 2 == 0:
                nc.vector.memset(qzt[j][64:128, :], 0.0)
            else:
                nc.vector.memset(qzt[j][0:64, :], 0.0)

        # ex double-buffered unit: 16 k-tiles x [128, 1024] bf16
        ex = [[ex_pool.tile([128, QW], BF16, tag="ex", bufs=2 * NLT,
                            name=f"ex{u}_{k}") for k in range(NLT)]
              for u in range(2)]

        # attn [q, d] per pair: 16 q-tiles x [128, 128] bf16
        # two pairs of attn [q,d] tiles live at once (write m+1 / transpose m)
        atp2 = [[at_pool.tile([128, 128], BF16, tag="atp", bufs=2 * NLT,
                              name=f"atp{mm}_{qg}") for qg in range(NLT)]
                for mm in range(2)]
        atp = [atp2[m % 2] for m in range(NM)]
        # attnT per pair: [128 (2 heads x 64), L] bf16
        attnT = [at_pool.tile([128, L], BF16, tag="attnT", bufs=NM,
                              name=f"attnT{m}") for m in range(NM)]

        # ---------- helpers ----------
        def v_prefetch(k):
            xs = []
            for i in range(NI):
                t = x_pool.tile([128, 128], BF16, tag="xvc", bufs=16,
                                name="xvc")
                nc.gpsimd.dma_start(
                    out=t, in_=xtv[i * 128:(i + 1) * 128,
                                   k * 128:(k + 1) * 128])
                xs.append(t)
            return xs

        def v_round(k, xs):
            """V projection for k-tile k -> vext[k] (ACT copy).
            Returns two sub-emissions (4+4 matmuls) for fine interleaving."""
            box = []

            def sub1():
                ps = pc_pool.tile([128, OC], F32, tag="pc", name="psv")
                box.append(ps)
                for i in range(4):
                    nc.tensor.matmul(ps, lhsT=xs[i], rhs=wv_t[i],
                                     start=(i == 0), stop=False)

            def sub2():
                ps = box[0]
                for i in range(4, NI):
                    nc.tensor.matmul(ps, lhsT=xs[i], rhs=wv_t[i],
                                     start=False, stop=(i == NI - 1))
                nc.scalar.copy(
                    vext[k][:, :, 0:DV],
                    ps.rearrange("p (h d) -> p h d", h=HPC))
            return [sub1, sub2]

        def qk_prefetch(which, lc):
            xsrc = xtq if which == "q" else xtk
            xs = []
            for i in range(NI):
                t = x_pool.tile([128, 512], BF16, tag="xc", bufs=16,
                                name="xc")
                nc.gpsimd.dma_start(
                    out=t, in_=xsrc[i * 128:(i + 1) * 128,
                                    lc * 512:(lc + 1) * 512])
                xs.append(t)
            return xs

        def qk_round(m, which, lc, xs):
            """Q or K projection for pair m, l-chunk lc (512 cols).
            Returns two sub-emissions (4+4 matmuls). Q output is split
            into the two heads' zero-padded ring tiles."""
            w_t = wq_t if which == "q" else wk_t
            box = []

            def sub1():
                ps = pc_pool.tile([128, 512], F32, tag="pc", name="psp")
                box.append(ps)
                for i in range(4):
                    nc.tensor.matmul(ps,
                                     lhsT=w_t[i][:, m * 128:(m + 1) * 128],
                                     rhs=xs[i], start=(i == 0), stop=False)

            def sub2():
                ps = box[0]
                for i in range(4, NI):
                    nc.tensor.matmul(ps,
                                     lhsT=w_t[i][:, m * 128:(m + 1) * 128],
                                     rhs=xs[i], start=False,
                                     stop=(i == NI - 1))
                sl = slice(lc * 512, (lc + 1) * 512)
                if which == "q":
                    qe = qzt[(2 * m) % 4]
                    qo = qzt[(2 * m + 1) % 4]
                    nc.vector.tensor_scalar_add(
                        qe[0:64, sl], ps[0:64, :], bq_t[0:64, m:m + 1])
                    nc.vector.tensor_scalar_add(
                        qo[64:128, sl], ps[64:128, :], bq_t[64:128, m:m + 1])
                else:
                    nc.vector.tensor_copy(kt[m][:, sl], ps)
            return [sub1, sub2]

        def transpose_pair(m):
            for qg in range(NLT):
                ps = pc_pool.tile([128, 512], F32, tag="pc", name="pst")
                tp = ps.bitcast(BF16)[:, 0:128]
                nc.tensor.matmul(tp, lhsT=atp[m][qg], rhs=ident,
                                 is_transpose=True, start=True, stop=True)
                nc.vector.tensor_copy(attnT[m][:, qg * 128:(qg + 1) * 128],
                                      tp)

        # work queue interleaved into stage B: (prefetch_fn, make_fn) items.
        # Each item expands to 2 sub-emissions (4-matmul chains) so the PE
        # never runs more than ~1us of filler between scores matmuls; the
        # head item's DMAs are issued one item ahead of its matmuls.
        # Order: pair-0 leftovers + V rounds (unit 0, deadline-ordered),
        # then pairs 1-3 spread evenly (pair m done before unit 4m).
        def qk_item(m, which, lc):
            return (lambda: qk_prefetch(which, lc),
                    lambda xs: qk_round(m, which, lc, xs))

        work = deque()
        work.append(qk_item(0, "k", 1))
        for k in range(6):
            work.append((lambda k=k: v_prefetch(k),
                         lambda xs, k=k: v_round(k, xs)))
        work.append(qk_item(0, "k", 2))
        for k in range(6, 12):
            work.append((lambda k=k: v_prefetch(k),
                         lambda xs, k=k: v_round(k, xs)))
        work.append(qk_item(0, "k", 3))
        for k in range(12, NLT):
            work.append((lambda k=k: v_prefetch(k),
                         lambda xs, k=k: v_round(k, xs)))
        work.append(qk_item(0, "q", 2))
        work.append(qk_item(0, "q", 3))
        for m in range(1, NM):
            for lc in range(4):
                work.append(qk_item(m, "k", lc))
            for lc in range(4):
                work.append(qk_item(m, "q", lc))
        work_xs = [None]   # operands already prefetched for the head item
        work_subs = []     # pending sub-emissions of the current item

        def emit_work():
            if not work_subs and work:
                pf, make = work.popleft()
                xs = work_xs[0]
                if xs is None:
                    xs = pf()
                work_xs[0] = work[0][0]() if work else None
                work_subs.extend(make(xs))
            if work_subs:
                work_subs.pop(0)()

        def flush_subs():
            # finish any half-emitted item so its psum accumulation group
            # is closed before another pc-pool user (transpose/stage C)
            # can land on the same bank
            while work_subs:
                work_subs.pop(0)()
        # units 0..15; per-unit work quota in SUB-emission units (2 per
        # item; 90 total). Unit 0 takes the pair-0 leftovers + all V rounds
        # (deadline-ordered in the queue); pairs 1-3 are spread evenly so
        # the PE keeps enough density to hold its clock up, with pair m
        # complete before unit 4m starts.
        quota = [42, 6, 5, 5, 4, 4, 4, 4, 4, 4, 4, 4, 0, 0, 0, 0]

        # ---------- stage A: minimal pair-0 projections (k-lc0, q-lc0/1) --
        a_items = [("k", 0), ("q", 0), ("q", 1)]
        a_xs = deque()
        a_xs.append(qk_prefetch(*a_items[0]))
        a_xs.append(qk_prefetch(*a_items[1]))
        for idx, (which, lc) in enumerate(a_items):
            for sub in qk_round(0, which, lc, a_xs.popleft()):
                sub()
            if idx + 2 < len(a_items):
                a_xs.append(qk_prefetch(*a_items[idx + 2]))
            elif idx + 2 == len(a_items):
                work_xs[0] = work[0][0]() if work else None

        # ---------- stage B ----------
        def transpose_one(m, qg):
            ps = pc_pool.tile([128, 512], F32, tag="pc", name="pst")
            tp = ps.bitcast(BF16)[:, 0:128]
            nc.tensor.matmul(tp, lhsT=atp[m][qg], rhs=ident,
                             is_transpose=True, start=True, stop=True)
            nc.vector.tensor_copy(attnT[m][:, qg * 128:(qg + 1) * 128], tp)

        def c_tile(lt):
            """Output projection for l-tile lt (pc-pool chains + ACT copy)."""
            ob = ob_pool.tile([128, D], F32, tag="ob", name="ob")
            for nck in range(2):
                ps = pc_pool.tile([128, 512], F32, tag="pc", name="psc")
                for dt in range(NM):
                    nc.tensor.matmul(
                        ps,
                        lhsT=attnT[dt][:, lt * 128:(lt + 1) * 128],
                        rhs=wc_t[dt][:, nck * 512:(nck + 1) * 512],
                        start=(dt == 0), stop=(dt == NM - 1))
                nc.scalar.copy(ob[:, nck * 512:(nck + 1) * 512], ps)
            nc.sync.dma_start(out=out[lt * 128:(lt + 1) * 128, :], in_=ob)

        def av_chain_parts(v, q):
            """AV accumulation chain for unit v, local q-tile q, as two
            8-matmul halves so the PE never blocks the scores->exp edge
            for more than ~0.5us. One [128,65] psum accumulator."""
            m, h2, qc = v // 4, (v // 2) % 2, v % 2
            off = h2 * DV
            hh = m * 2 + h2
            exv = ex[v % 2]
            box = []

            def partA():
                av = av_pool.tile([128, DV + 1], F32, tag="av", name="av")
                box.append(av)
                for k in range(8):
                    nc.tensor.matmul(
                        av, lhsT=exv[k][:, q * 128:(q + 1) * 128],
                        rhs=vext[k][:, hh, :],
                        start=(k == 0), stop=False)

            def partB():
                av = box[0]
                for k in range(8, NLT):
                    nc.tensor.matmul(
                        av, lhsT=exv[k][:, q * 128:(q + 1) * 128],
                        rhs=vext[k][:, hh, :],
                        start=False, stop=(k == NLT - 1))
                qg = qc * NQT + q
                rc = ob_pool.tile([128, 1], F32, tag="rc", name="rc")
                nc.vector.reciprocal(rc, av[:, DV:DV + 1])
                nc.vector.tensor_scalar_mul(
                    atp[m][qg][:, off:off + DV], av[:, 0:DV], rc)
            return [partA, partB]

        for u in range(16):
            m, h2, qc = u // 4, (u // 2) % 2, u % 2
            off = h2 * DV
            exu = ex[u % 2]

            av_parts = deque()
            if u >= 1:
                for q in range(NQT):
                    av_parts.extend(av_chain_parts(u - 1, q))

            nwork = quota[u]
            emitted = 0

            def maybe_work(frac):
                nonlocal emitted
                want = int(frac * nwork + 0.5)
                while emitted < want and work:
                    emit_work()
                    emitted += 1

            qz = qzt[(2 * m + h2) % 4]
            for k in range(NLT):
                st = st_pool.tile([128, QW], F32, tag="st", name="st")
                for j in range(2):
                    nc.tensor.matmul(
                        st[:, j * 512:(j + 1) * 512],
                        lhsT=kt[m][:, k * 128:(k + 1) * 128],
                        rhs=qz[:, qc * QW + j * 512:qc * QW + (j + 1) * 512],
                        start=True, stop=True)
                nc.scalar.activation(out=exu[k], in_=st, func=EXP,
                                     scale=0.125)
                maybe_work((k + 1) / NLT)
                # AV chain halves of the previous unit overlap this
                # unit's exps, one half per k-iteration
                if av_parts:
                    av_parts.popleft()()
                if u == 15 and k >= 3 and k % 2 == 1 and (k - 3) // 2 <= 6:
                    # pair-3 q-tiles 0..6 complete as unit-14's chains land:
                    # transpose them and emit their output-projection tiles
                    # early, shrinking the serial tail
                    qg = (k - 3) // 2
                    transpose_one(3, qg)
                    c_tile(qg)
            while av_parts:
                av_parts.popleft()()
            if u % 4 == 0 and u >= 4:
                flush_subs()
                transpose_pair(u // 4 - 1)
        # drain: transpose qg=7, then unit-15 chains with their transposes
        # and remaining output tiles interleaved
        transpose_one(3, 7)
        c_tile(7)
        for q in range(NQT):
            for part in av_chain_parts(15, q):
                part()
            transpose_one(3, NQT + q)
            c_tile(NQT + q)
        stack.close()

    nc.compile()
    return nc


def _get_nc():
    if "nc" not in _CACHE:
        _CACHE["nc"] = _build()
    return _CACHE["nc"]


def kernel(query, key, value, Wq, bq, Wk, bk, Wv, bv, Wc, bc, **_unused):
    import ml_dtypes
    bf16 = ml_dtypes.bfloat16

    query = np.asarray(query, np.float32)
    key = np.asarray(key, np.float32)
    value = np.asarray(value, np.float32)
    Wq = np.asarray(Wq, np.float32)
    Wk = np.asarray(Wk, np.float32)
    Wv = np.asarray(Wv, np.float32)
    Wc = np.asarray(Wc, np.float32)
    bq = np.asarray(bq, np.float32)
    bk = np.asarray(bk, np.float32)
    bv = np.asarray(bv, np.float32)
    bc = np.asarray(bc, np.float32)

    nc = _get_nc()

    # bk folded out exactly (softmax col-invariance); bv folded into a
    # constant output row: out += bv @ Wc.T + bc
    const_row = (bv @ Wc.T + bc).astype(np.float32)

    xtq = [np.ascontiguousarray(query[b].T).astype(bf16) for b in range(B)]
    xtk = [np.ascontiguousarray(key[b].T).astype(bf16) for b in range(B)]
    xtv = [np.ascontiguousarray(value[b].T).astype(bf16) for b in range(B)]
    wqt_g = [np.ascontiguousarray(Wq[g * OC:(g + 1) * OC, :].T).astype(bf16)
             for g in range(2)]
    wkt_g = [np.ascontiguousarray(Wk[g * OC:(g + 1) * OC, :].T).astype(bf16)
             for g in range(2)]
    wvt_g = [np.ascontiguousarray(Wv[g * OC:(g + 1) * OC, :].T).astype(bf16)
             for g in range(2)]
    wct_g = [np.ascontiguousarray(Wc[:, g * OC:(g + 1) * OC].T).astype(bf16)
             for g in range(2)]
    ident = np.eye(128, dtype=np.float32).astype(bf16)

    in_maps = []
    for c in range(NCORES):
        b, g = c // 2, c % 2
        in_maps.append({
            "XTQ": xtq[b], "XTK": xtk[b], "XTV": xtv[b],
            "WQT": wqt_g[g], "WKT": wkt_g[g], "WVT": wvt_g[g],
            "WCT": wct_g[g], "IDT": ident,
            "BQ": np.ascontiguousarray(bq[g * OC:(g + 1) * OC]),
        })

    res = run_bass_kernel_spmd(nc, in_maps, core_ids=list(range(NCORES)),
                               **_CACHE.get("run_kwargs", {}))
    _CACHE["last_results"] = res

    outp = np.empty((B, L, D), np.float32)
    for b in range(B):
        outp[b] = res.results[2 * b]["OUT"] + res.results[2 * b + 1]["OUT"]
    outp += const_row
    return outp


# revision 6
# speedup vs baseline: 1.1843x; 1.1843x over previous
"""Multi-head attention forward on 8 Trainium2 NeuronCores (Bass/Tile), v2.

Problem: B=4, L=2048, D=1024, H=16 heads, DV=64.
  out = softmax((x_q Wq^T + bq)(x_k Wk^T)^T / sqrt(DV)) (x_v Wv^T) Wc^T + const
(bk dropped exactly via softmax column-invariance; bv/bc folded into a
host-side constant row added after the kernel.)

Sharding (8 cores): core c handles batch b = c//2 and head-group g = c%2
(8 heads). Host sums the two per-batch partials and adds the constant row.

All matmuls bf16 (full-rate on the PE regardless of K=64 / M=65 / N=65,
unlike fp32r). Per core:
  A. K/Q projections for pair 0 upfront; V projection and the remaining
     Q/K pairs are interleaved into stage B to keep the PE dense while
     the ACT engine (the near-bottleneck, ~340us of exp) drains scores.
  B. Per unit (pair m, head h2, q-half qc): 16 k-tiles: scores^T
     [k=128, q=1024] in PSUM -> ACT exp (scale=1/8) -> ex bf16; AV in
     [q, d] orientation: lhsT=ex slice [128,128], rhs=vext [128,65]
     (65th col = ones -> softmax denominator in PSUM col 64), N=65
     matmuls accumulate over k into packed PSUM banks (4 q-tiles x 65
     cols per bank). Normalize: per-partition reciprocal + scale on DVE.
     At pair completion: PE-transpose attn [q,d] -> attnT [d,q] bf16.
  C. Output projection from attnT (N=512 matmuls), ACT copies, DMA out.
"""

from collections import deque
from contextlib import ExitStack

import numpy as np

import concourse.bacc as bacc
import concourse.mybir as mybir
from concourse.tile import TileContext
from concourse.bass_utils import run_bass_kernel_spmd

B, L, D, H = 4, 2048, 1024, 16
DV = 64
HPC = 8           # heads per core
OC = HPC * DV     # 512 projection cols per core
NCORES = 8

F32 = mybir.dt.float32
BF16 = mybir.dt.bfloat16
EXP = mybir.ActivationFunctionType.Exp

NI = D // 128     # 8 contraction tiles for projections
NM = HPC // 2     # 4 head pairs per core
NLT = L // 128    # 16 l/k tiles
NQT = 8           # q-tiles per q-half unit
QW = 1024         # q-half width

_CACHE = {}


def _build():
    nc = bacc.Bacc("TRN2", target_bir_lowering=False, debug=False,
                   num_devices=NCORES)

    xtq = nc.dram_tensor("XTQ", [D, L], BF16, kind="ExternalInput")
    xtk = nc.dram_tensor("XTK", [D, L], BF16, kind="ExternalInput")
    xtv = nc.dram_tensor("XTV", [D, L], BF16, kind="ExternalInput")
    wqt = nc.dram_tensor("WQT", [D, OC], BF16, kind="ExternalInput")
    wkt = nc.dram_tensor("WKT", [D, OC], BF16, kind="ExternalInput")
    wvt = nc.dram_tensor("WVT", [D, OC], BF16, kind="ExternalInput")
    wct = nc.dram_tensor("WCT", [OC, D], BF16, kind="ExternalInput")
    bqd = nc.dram_tensor("BQ", [OC], F32, kind="ExternalInput")
    idt = nc.dram_tensor("IDT", [128, 128], BF16, kind="ExternalInput")
    out = nc.dram_tensor("OUT", [L, D], BF16, kind="ExternalOutput")

    with TileContext(nc) as tc:
        stack = ExitStack()
        w_pool = stack.enter_context(tc.tile_pool(name="w", bufs=1))
        x_pool = stack.enter_context(tc.tile_pool(name="x", bufs=1))
        qk_pool = stack.enter_context(tc.tile_pool(name="qk", bufs=1))
        ex_pool = stack.enter_context(tc.tile_pool(name="ex", bufs=1))
        at_pool = stack.enter_context(tc.tile_pool(name="at", bufs=1))
        ob_pool = stack.enter_context(tc.tile_pool(name="ob", bufs=2))
        st_pool = stack.enter_context(tc.tile_pool(name="st", bufs=2,
                                                   space="PSUM"))
        av_pool = stack.enter_context(tc.tile_pool(name="av", bufs=2,
                                                   space="PSUM"))
        pc_pool = stack.enter_context(tc.tile_pool(name="pc", bufs=2,
                                                   space="PSUM"))

        # --- static SBUF tensors (K/Q weights first: stage A needs them) ---
        wv_t, wq_t, wk_t = [], [], []
        for i in range(NI):
            t = w_pool.tile([128, OC], BF16, tag="wk", bufs=NI, name="wk_t")
            nc.sync.dma_start(out=t, in_=wkt[i * 128:(i + 1) * 128, :])
            wk_t.append(t)
        for i in range(NI):
            t = w_pool.tile([128, OC], BF16, tag="wq", bufs=NI, name="wq_t")
            nc.sync.dma_start(out=t, in_=wqt[i * 128:(i + 1) * 128, :])
            wq_t.append(t)
        bq_t = w_pool.tile([128, NM], F32, tag="bq")
        nc.sync.dma_start(out=bq_t,
                          in_=bqd[:].rearrange("(m p) -> p m", p=128))
        for i in range(NI):
            t = w_pool.tile([128, OC], BF16, tag="wv", bufs=NI, name="wv_t")
            nc.sync.dma_start(out=t, in_=wvt[i * 128:(i + 1) * 128, :])
            wv_t.append(t)
        ident = w_pool.tile([128, 128], BF16, tag="idt")
        nc.sync.dma_start(out=ident, in_=idt[:, :])
        wc_t = []
        for dt in range(NM):
            t = w_pool.tile([128, D], BF16, tag="wc", bufs=NM, name="wc_t")
            nc.sync.dma_start(out=t, in_=wct[dt * 128:(dt + 1) * 128, :])
            wc_t.append(t)

        # vext: [128(k), 8 heads, 65] bf16, ones in col 64 of each head
        vext = [x_pool.tile([128, HPC, DV + 1], BF16, tag="vext", bufs=NLT,
                            name=f"vext{k}") for k in range(NLT)]
        for k in range(NLT):
            nc.vector.memset(vext[k][:, :, DV], 1.0)

        # kt per pair: [128 (2 heads x 64 dv), L] bf16.
        # Q lives in a 4-slot ring of ZERO-PADDED per-head tiles: head h
        # occupies its own 64 rows (h%2 parity decides which half), the
        # other 64 rows stay zero. Scores then contract over K=128 with
        # the full kt tile as lhsT -- the zero rows contribute nothing,
        # and the full-K shape keeps the PE clock at 2.4 GHz (half-K
        # matmul streams throttle the PE to 1.2 GHz).
        kt = [qk_pool.tile([128, L], BF16, tag="qkt", bufs=NM,
                           name=f"kt{m}") for m in range(NM)]
        qzt = [qk_pool.tile([128, L], BF16, tag="qzt", bufs=4,
                            name=f"qzt{j}") for j in range(4)]
        for j in range(4):
            if j % 2 == 0:
                nc.vector.memset(qzt[j][64:128, :], 0.0)
            else:
                nc.vector.memset(qzt[j][0:64, :], 0.0)

        # ex double-buffered unit: 16 k-tiles x [128, 1024] bf16
        ex = [[ex_pool.tile([128, QW], BF16, tag="ex", bufs=2 * NLT,
                            name=f"ex{u}_{k}") for k in range(NLT)]
              for u in range(2)]

        # attn [q, d] per pair: 16 q-tiles x [128, 128] bf16
        # two pairs of attn [q,d] tiles live at once (write m+1 / transpose m)
        atp2 = [[at_pool.tile([128, 128], BF16, tag="atp", bufs=2 * NLT,
                              name=f"atp{mm}_{qg}") for qg in range(NLT)]
                for mm in range(2)]
        atp = [atp2[m % 2] for m in range(NM)]
        # attnT per pair: [128 (2 heads x 64), L] bf16
        attnT = [at_pool.tile([128, L], BF16, tag="attnT", bufs=NM,
                              name=f"attnT{m}") for m in range(NM)]

        # ---------- helpers ----------
        def v_prefetch(k):
            xs = []
            for i in range(NI):
                t = x_pool.tile([128, 128], BF16, tag="xvc", bufs=16,
                                name="xvc")
                nc.gpsimd.dma_start(
                    out=t, in_=xtv[i * 128:(i + 1) * 128,
                                   k * 128:(k + 1) * 128])
                xs.append(t)
            return xs

        def v_round(k, xs):
            """V projection for k-tile k -> vext[k] (ACT copy).
            Returns two sub-emissions (4+4 matmuls) for fine interleaving."""
            box = []

            def sub1():
                ps = pc_pool.tile([128, OC], F32, tag="pc", name="psv")
                box.append(ps)
                for i in range(4):
                    nc.tensor.matmul(ps, lhsT=xs[i], rhs=wv_t[i],
                                     start=(i == 0), stop=False)

            def sub2():
                ps = box[0]
                for i in range(4, NI):
                    nc.tensor.matmul(ps, lhsT=xs[i], rhs=wv_t[i],
                                     start=False, stop=(i == NI - 1))
                nc.scalar.copy(
                    vext[k][:, :, 0:DV],
                    ps.rearrange("p (h d) -> p h d", h=HPC))
            return [sub1, sub2]

        def qk_prefetch(which, lc):
            xsrc = xtq if which == "q" else xtk
            xs = []
            for i in range(NI):
                t = x_pool.tile([128, 512], BF16, tag="xc", bufs=16,
                                name="xc")
                nc.gpsimd.dma_start(
                    out=t, in_=xsrc[i * 128:(i + 1) * 128,
                                    lc * 512:(lc + 1) * 512])
                xs.append(t)
            return xs

        def qk_round(m, which, lc, xs):
            """Q or K projection for pair m, l-chunk lc (512 cols).
            Returns two sub-emissions (4+4 matmuls). Q output is split
            into the two heads' zero-padded ring tiles."""
            w_t = wq_t if which == "q" else wk_t
            box = []

            def sub1():
                ps = pc_pool.tile([128, 512], F32, tag="pc", name="psp")
                box.append(ps)
                for i in range(4):
                    nc.tensor.matmul(ps,
                                     lhsT=w_t[i][:, m * 128:(m + 1) * 128],
                                     rhs=xs[i], start=(i == 0), stop=False)

            def sub2():
                ps = box[0]
                for i in range(4, NI):
                    nc.tensor.matmul(ps,
                                     lhsT=w_t[i][:, m * 128:(m + 1) * 128],
                                     rhs=xs[i], start=False,
                                     stop=(i == NI - 1))
                sl = slice(lc * 512, (lc + 1) * 512)
                if which == "q":
                    qe = qzt[(2 * m) % 4]
                    qo = qzt[(2 * m + 1) % 4]
                    nc.vector.tensor_scalar_add(
                        qe[0:64, sl], ps[0:64, :], bq_t[0:64, m:m + 1])
                    nc.vector.tensor_scalar_add(
                        qo[64:128, sl], ps[64:128, :], bq_t[64:128, m:m + 1])
                else:
                    nc.vector.tensor_copy(kt[m][:, sl], ps)
            return [sub1, sub2]

        def transpose_pair(m):
            for qg in range(NLT):
                ps = pc_pool.tile([128, 512], F32, tag="pc", name="pst")
                tp = ps.bitcast(BF16)[:, 0:128]
                nc.tensor.matmul(tp, lhsT=atp[m][qg], rhs=ident,
                                 is_transpose=True, start=True, stop=True)
                nc.vector.tensor_copy(attnT[m][:, qg * 128:(qg + 1) * 128],
                                      tp)

        # work queue interleaved into stage B: (prefetch_fn, make_fn) items.
        # Each item expands to 2 sub-emissions (4-matmul chains) so the PE
        # never runs more than ~1us of filler between scores matmuls; the
        # head item's DMAs are issued one item ahead of its matmuls.
        # Order: pair-0 leftovers + V rounds (unit 0, deadline-ordered),
        # then pairs 1-3 spread evenly (pair m done before unit 4m).
        def qk_item(m, which, lc):
            return (lambda: qk_prefetch(which, lc),
                    lambda xs: qk_round(m, which, lc, xs))

        work = deque()
        work.append(qk_item(0, "k", 1))
        for k in range(6):
            work.append((lambda k=k: v_prefetch(k),
                         lambda xs, k=k: v_round(k, xs)))
        work.append(qk_item(0, "k", 2))
        for k in range(6, 12):
            work.append((lambda k=k: v_prefetch(k),
                         lambda xs, k=k: v_round(k, xs)))
        work.append(qk_item(0, "k", 3))
        for k in range(12, NLT):
            work.append((lambda k=k: v_prefetch(k),
                         lambda xs, k=k: v_round(k, xs)))
        work.append(qk_item(0, "q", 2))
        work.append(qk_item(0, "q", 3))
        for m in range(1, NM):
            for lc in range(4):
                work.append(qk_item(m, "k", lc))
            for lc in range(4):
                work.append(qk_item(m, "q", lc))
        work_xs = [None]   # operands already prefetched for the head item
        work_subs = []     # pending sub-emissions of the current item

        def emit_work():
            if not work_subs and work:
                pf, make = work.popleft()
                xs = work_xs[0]
                if xs is None:
                    xs = pf()
                work_xs[0] = work[0][0]() if work else None
                work_subs.extend(make(xs))
            if work_subs:
                work_subs.pop(0)()

        def flush_subs():
            # finish any half-emitted item so its psum accumulation group
            # is closed before another pc-pool user (transpose/stage C)
            # can land on the same bank
            while work_subs:
                work_subs.pop(0)()
        # units 0..15; per-unit work quota in SUB-emission units (2 per
        # item; 90 total). Unit 0 takes the pair-0 leftovers + all V rounds
        # (deadline-ordered in the queue); pairs 1-3 are spread evenly so
        # the PE keeps enough density to hold its clock up, with pair m
        # complete before unit 4m starts.
        quota = [42, 6, 5, 5, 4, 4, 4, 4, 4, 4, 4, 4, 0, 0, 0, 0]

        # ---------- stage A: minimal pair-0 projections (k-lc0, q-lc0/1) --
        a_items = [("k", 0), ("q", 0), ("q", 1)]
        a_xs = deque()
        a_xs.append(qk_prefetch(*a_items[0]))
        a_xs.append(qk_prefetch(*a_items[1]))
        for idx, (which, lc) in enumerate(a_items):
            for sub in qk_round(0, which, lc, a_xs.popleft()):
                sub()
            if idx + 2 < len(a_items):
                a_xs.append(qk_prefetch(*a_items[idx + 2]))
            elif idx + 2 == len(a_items):
                work_xs[0] = work[0][0]() if work else None

        # ---------- stage B ----------
        def transpose_one(m, qg):
            ps = pc_pool.tile([128, 512], F32, tag="pc", name="pst")
            tp = ps.bitcast(BF16)[:, 0:128]
            nc.tensor.matmul(tp, lhsT=atp[m][qg], rhs=ident,
                             is_transpose=True, start=True, stop=True)
            nc.vector.tensor_copy(attnT[m][:, qg * 128:(qg + 1) * 128], tp)

        def c_tile(lt):
            """Output projection for l-tile lt (pc-pool chains + ACT copy)."""
            ob = ob_pool.tile([128, D], BF16, tag="ob", name="ob")
            for nck in range(2):
                ps = pc_pool.tile([128, 512], F32, tag="pc", name="psc")
                for dt in range(NM):
                    nc.tensor.matmul(
                        ps,
                        lhsT=attnT[dt][:, lt * 128:(lt + 1) * 128],
                        rhs=wc_t[dt][:, nck * 512:(nck + 1) * 512],
                        start=(dt == 0), stop=(dt == NM - 1))
                nc.scalar.copy(ob[:, nck * 512:(nck + 1) * 512], ps)
            nc.sync.dma_start(out=out[lt * 128:(lt + 1) * 128, :], in_=ob)

        def av_chain_parts(v, q):
            """AV accumulation chain for unit v, local q-tile q, as two
            8-matmul halves so the PE never blocks the scores->exp edge
            for more than ~0.5us. One [128,65] psum accumulator."""
            m, h2, qc = v // 4, (v // 2) % 2, v % 2
            off = h2 * DV
            hh = m * 2 + h2
            exv = ex[v % 2]
            box = []

            def partA():
                av = av_pool.tile([128, DV + 1], F32, tag="av", name="av")
                box.append(av)
                for k in range(8):
                    nc.tensor.matmul(
                        av, lhsT=exv[k][:, q * 128:(q + 1) * 128],
                        rhs=vext[k][:, hh, :],
                        start=(k == 0), stop=False)

            def partB():
                av = box[0]
                for k in range(8, NLT):
                    nc.tensor.matmul(
                        av, lhsT=exv[k][:, q * 128:(q + 1) * 128],
                        rhs=vext[k][:, hh, :],
                        start=False, stop=(k == NLT - 1))
                qg = qc * NQT + q
                rc = ob_pool.tile([128, 1], F32, tag="rc", name="rc")
                nc.vector.reciprocal(rc, av[:, DV:DV + 1])
                nc.vector.tensor_scalar_mul(
                    atp[m][qg][:, off:off + DV], av[:, 0:DV], rc)
            return [partA, partB]

        for u in range(16):
            m, h2, qc = u // 4, (u // 2) % 2, u % 2
            off = h2 * DV
            exu = ex[u % 2]

            av_parts = deque()
            if u >= 1:
                for q in range(NQT):
                    av_parts.extend(av_chain_parts(u - 1, q))

            nwork = quota[u]
            emitted = 0

            def maybe_work(frac):
                nonlocal emitted
                want = int(frac * nwork + 0.5)
                while emitted < want and work:
                    emit_work()
                    emitted += 1

            qz = qzt[(2 * m + h2) % 4]
            for k in range(NLT):
                st = st_pool.tile([128, QW], F32, tag="st", name="st")
                for j in range(2):
                    nc.tensor.matmul(
                        st[:, j * 512:(j + 1) * 512],
                        lhsT=kt[m][:, k * 128:(k + 1) * 128],
                        rhs=qz[:, qc * QW + j * 512:qc * QW + (j + 1) * 512],
                        start=True, stop=True)
                nc.scalar.activation(out=exu[k], in_=st, func=EXP,
                                     scale=0.125)
                maybe_work((k + 1) / NLT)
                # AV chain halves of the previous unit overlap this
                # unit's exps, one half per k-iteration
                if av_parts:
                    av_parts.popleft()()
                if u == 15 and k >= 3 and k % 2 == 1 and (k - 3) // 2 <= 6:
                    # pair-3 q-tiles 0..6 complete as unit-14's chains land:
                    # transpose them and emit their output-projection tiles
                    # early, shrinking the serial tail
                    qg = (k - 3) // 2
                    transpose_one(3, qg)
                    c_tile(qg)
            while av_parts:
                av_parts.popleft()()
            if u % 4 == 0 and u >= 4:
                flush_subs()
                transpose_pair(u // 4 - 1)
        # drain: transpose qg=7, then unit-15 chains with their transposes
        # and remaining output tiles interleaved
        transpose_one(3, 7)
        c_tile(7)
        for q in range(NQT):
            for part in av_chain_parts(15, q):
                part()
            transpose_one(3, NQT + q)
            c_tile(NQT + q)
        stack.close()

    nc.compile()
    return nc


def _get_nc():
    if "nc" not in _CACHE:
        _CACHE["nc"] = _build()
    return _CACHE["nc"]


def kernel(query, key, value, Wq, bq, Wk, bk, Wv, bv, Wc, bc, **_unused):
    import ml_dtypes
    bf16 = ml_dtypes.bfloat16

    query = np.asarray(query, np.float32)
    key = np.asarray(key, np.float32)
    value = np.asarray(value, np.float32)
    Wq = np.asarray(Wq, np.float32)
    Wk = np.asarray(Wk, np.float32)
    Wv = np.asarray(Wv, np.float32)
    Wc = np.asarray(Wc, np.float32)
    bq = np.asarray(bq, np.float32)
    bk = np.asarray(bk, np.float32)
    bv = np.asarray(bv, np.float32)
    bc = np.asarray(bc, np.float32)

    nc = _get_nc()

    # bk folded out exactly (softmax col-invariance); bv folded into a
    # constant output row: out += bv @ Wc.T + bc
    const_row = (bv @ Wc.T + bc).astype(np.float32)

    xtq = [np.ascontiguousarray(query[b].T).astype(bf16) for b in range(B)]
    xtk = [np.ascontiguousarray(key[b].T).astype(bf16) for b in range(B)]
    xtv = [np.ascontiguousarray(value[b].T).astype(bf16) for b in range(B)]
    wqt_g = [np.ascontiguousarray(Wq[g * OC:(g + 1) * OC, :].T).astype(bf16)
             for g in range(2)]
    wkt_g = [np.ascontiguousarray(Wk[g * OC:(g + 1) * OC, :].T).astype(bf16)
             for g in range(2)]
    wvt_g = [np.ascontiguousarray(Wv[g * OC:(g + 1) * OC, :].T).astype(bf16)
             for g in range(2)]
    wct_g = [np.ascontiguousarray(Wc[:, g * OC:(g + 1) * OC].T).astype(bf16)
             for g in range(2)]
    ident = np.eye(128, dtype=np.float32).astype(bf16)

    in_maps = []
    for c in range(NCORES):
        b, g = c // 2, c % 2
        in_maps.append({
            "XTQ": xtq[b], "XTK": xtk[b], "XTV": xtv[b],
            "WQT": wqt_g[g], "WKT": wkt_g[g], "WVT": wvt_g[g],
            "WCT": wct_g[g], "IDT": ident,
            "BQ": np.ascontiguousarray(bq[g * OC:(g + 1) * OC]),
        })

    res = run_bass_kernel_spmd(nc, in_maps, core_ids=list(range(NCORES)),
                               **_CACHE.get("run_kwargs", {}))
    _CACHE["last_results"] = res

    outp = np.empty((B, L, D), np.float32)
    for b in range(B):
        outp[b] = (np.asarray(res.results[2 * b]["OUT"], np.float32)
                   + np.asarray(res.results[2 * b + 1]["OUT"], np.float32))
    outp += const_row
    return outp
